# revision 1
# baseline (speedup 1.0000x reference)
"""Trainium2 Bass kernel for bidirectional masked-LSTM + attention pooling + FC head.

Problem (hardcoded shapes): B=64, T=512, E=256, H=512, OH=1024.
  - x [B,T,E] f32, lengths [B] i32, attn_w [T] f32
  - per-direction LSTM weights Wih [4H,E], Whh [4H,H], biases [4H]
  - fc1 [OH,2H]+[OH], fc2 [T,OH]+[T]
  - out: logits [B,T] f32, padded positions = -1e30

Sharding: 8 cores = 4 batch groups (16 seqs) x 2 directions. Each core runs one
direction's full 512-step recurrence for its 16 sequences. Attention pooling is
folded into the recurrence as a masked weighted accumulate (per-(t,b) scale
table precomputed on host, which also implements sequence reversal masking for
the backward direction). The FC head runs on every core; forward/backward
pooled partials are combined with a pairwise AllReduce.

Layouts (per core):
  h "hidden-tiled" [128, K_CH*16]: h[b, hid] at partition hid%128, col (hid//128)*16+b.
  gates PSUM tiled [128, m*16+b] per gate-chunk m (gate g=m*128+p), gate order
  permuted to [i, f, o, g] so i,f share one sigmoid and g is one tanh.
"""

import os

import numpy as np

import concourse.bass as bass
import concourse.tile as tile
from concourse import bacc, mybir
from concourse.bass_utils import run_bass_kernel_spmd

B, T, E, H, OH = 64, 512, 256, 512, 1024
G = 4 * H          # 2048 gates
BL = 16            # batch per core
M_CH = G // 128    # 16 gate chunks
K_CH = H // 128    # 4 hidden chunks
E_CH = E // 128    # 2 input chunks
MO_CH = OH // 128  # 8
MT_CH = T // 128   # 4
NBLK = 32          # xp prefetch block (timesteps)

f32 = mybir.dt.float32
f16 = mybir.dt.float16
AF = mybir.ActivationFunctionType
ALU = mybir.AluOpType

# gate permutation: torch order [i,f,g,o] -> kernel order [i,f,o,g]
# perm[new_pos] = old_index  (applied to rows of Wih/Whh and bias)
_GPERM = np.concatenate([
    np.arange(0, H),          # i
    np.arange(H, 2 * H),      # f
    np.arange(3 * H, 4 * H),  # o
    np.arange(2 * H, 3 * H),  # g
])


def _bc_free(ap, reps, width):
    """AP that broadcasts a [P, width] slice to [P, reps, width] via stride-0."""
    return bass.AP(
        tensor=ap.tensor,
        offset=ap.offset,
        ap=[ap.ap[0], [0, reps]] + list(ap.ap[1:]),
    )


def build_nc(t_steps=T, use_collective=True):
    nc = bacc.Bacc("TRN2", target_bir_lowering=False, num_devices=8)

    # ---- DRAM parameters (per-core payloads prepared on host) ----
    xt = nc.declare_dram_parameter("xt", [E_CH, 128, BL * T], f16, isOutput=False)
    wih = nc.declare_dram_parameter("wih", [E_CH, 128, G], f16, isOutput=False)
    whh = nc.declare_dram_parameter("whh", [K_CH, 128, G], f16, isOutput=False)
    biasT = nc.declare_dram_parameter("biasT", [128, M_CH], f32, isOutput=False)
    sc = nc.declare_dram_parameter("sc", [128, T, BL], f16, isOutput=False)
    w1t = nc.declare_dram_parameter("w1t", [K_CH, 128, OH], f16, isOutput=False)
    b1T = nc.declare_dram_parameter("b1T", [128, MO_CH], f32, isOutput=False)
    w2t = nc.declare_dram_parameter("w2t", [MO_CH, 128, T], f16, isOutput=False)
    b2T = nc.declare_dram_parameter("b2T", [128, MT_CH], f32, isOutput=False)

    out_logits = nc.declare_dram_parameter("out_logits", [128, MT_CH * BL], f32,
                                           isOutput=True)
    out_pooled = nc.declare_dram_parameter("out_pooled", [128, K_CH * BL], f32,
                                           isOutput=True)

    xp_dram = nc.dram_tensor("xp_scratch", [M_CH, BL, 128, T], f16)
    ar_in = nc.dram_tensor("ar_in", [128, MO_CH * BL], f32)
    ar_out = nc.dram_tensor("ar_out", [128, MO_CH * BL], f32)

    with tile.TileContext(nc) as tc:
        with tc.tile_pool(name="const", bufs=1) as const_pool:
            whh_sb = const_pool.tile([128, K_CH, G], f16)
            for k in range(K_CH):
                nc.sync.dma_start(out=whh_sb[:, k, :], in_=whh[k])
            biasT_sb = const_pool.tile([128, M_CH], f32)
            nc.sync.dma_start(out=biasT_sb, in_=biasT[:, :])
            sc_sb = const_pool.tile([128, T, BL], f16)
            nc.sync.dma_start(out=sc_sb, in_=sc[:, :, :])
            w1t_sb = const_pool.tile([128, K_CH, OH], f16)
            for k in range(K_CH):
                nc.sync.dma_start(out=w1t_sb[:, k, :], in_=w1t[k])
            b1T_sb = const_pool.tile([128, MO_CH], f32)
            nc.sync.dma_start(out=b1T_sb, in_=b1T[:, :])
            w2t_sb = const_pool.tile([128, MO_CH, T], f16)
            for k in range(MO_CH):
                nc.sync.dma_start(out=w2t_sb[:, k, :], in_=w2t[k])
            b2T_sb = const_pool.tile([128, MT_CH], f32)
            nc.sync.dma_start(out=b2T_sb, in_=b2T[:, :])

            # ---- Phase 1: input projection xp = x @ Wih_perm.T + bias ----
            with tc.tile_pool(name="proj_in", bufs=1) as proj_in, \
                 tc.tile_pool(name="proj_ps", bufs=4, space="PSUM") as proj_ps, \
                 tc.tile_pool(name="proj_ev", bufs=4) as proj_ev:
                wih_sb = proj_in.tile([128, E_CH, G], f16)
                for k in range(E_CH):
                    nc.sync.dma_start(out=wih_sb[:, k, :], in_=wih[k])
                xt_sb = proj_in.tile([128, E_CH, BL * T], f16)
                for k in range(E_CH):
                    nc.sync.dma_start(out=xt_sb[:, k, :], in_=xt[k])

                for m in range(M_CH):
                    for b in range(BL):
                        ps = proj_ps.tile([128, 512], f32, tag="pp")
                        for k in range(E_CH):
                            nc.tensor.matmul(
                                ps,
                                lhsT=wih_sb[:, k, m * 128:(m + 1) * 128],
                                rhs=xt_sb[:, k, b * T:b * T + 512],
                                start=(k == 0),
                                stop=(k == E_CH - 1),
                            )
                        xp_sb = proj_ev.tile([128, 512], f16, tag="xps")
                        if (m * BL + b) % 2 == 0:
                            nc.vector.tensor_scalar(
                                out=xp_sb, in0=ps,
                                scalar1=biasT_sb[:, m:m + 1], scalar2=None,
                                op0=ALU.add,
                            )
                        else:
                            nc.scalar.activation(
                                out=xp_sb, in_=ps, func=AF.Identity,
                                bias=biasT_sb[:, m:m + 1], scale=1.0,
                            )
                        nc.sync.dma_start(out=xp_dram[m, b], in_=xp_sb)

            # ---- Phase 2: recurrence ----
            with tc.tile_pool(name="state", bufs=1) as state_pool:
                h_sb = state_pool.tile([128, K_CH * BL], f16)
                c_sb = state_pool.tile([128, K_CH * BL], f32)
                acc = state_pool.tile([128, K_CH * BL], f32)
                nc.vector.memset(h_sb, 0.0)
                nc.vector.memset(c_sb, 0.0)
                nc.gpsimd.memset(acc, 0.0)

                with tc.tile_pool(name="xpb", bufs=2) as xpb_pool, \
                     tc.tile_pool(name="rec_ps", bufs=2, space="PSUM") as rec_ps, \
                     tc.tile_pool(name="work", bufs=2) as work:
                    n_blocks = (t_steps + NBLK - 1) // NBLK
                    for blk in range(n_blocks):
                        t0 = blk * NBLK
                        nt = min(NBLK, t_steps - t0)
                        xpb = xpb_pool.tile([128, M_CH * BL, NBLK], f16, tag="xpb")
                        for m in range(M_CH):
                            for b in range(BL):
                                nc.sync.dma_start(
                                    out=xpb[:, m * BL + b, :nt],
                                    in_=xp_dram[m, b, :, t0:t0 + nt],
                                )
                        for tt in range(nt):
                            t = t0 + tt
                            # PE: group order g, i+f, o
                            psg = rec_ps.tile([128, 64], f32, tag="psg")
                            pif = rec_ps.tile([128, 128], f32, tag="psif")
                            pso = rec_ps.tile([128, 64], f32, tag="pso")
                            for j, m in enumerate(range(12, 16)):  # g
                                for k in range(K_CH):
                                    nc.tensor.matmul(
                                        psg[:, j * 16:(j + 1) * 16],
                                        lhsT=whh_sb[:, k, m * 128:(m + 1) * 128],
                                        rhs=h_sb[:, k * BL:(k + 1) * BL],
                                        start=(k == 0), stop=(k == K_CH - 1),
                                    )
                            gsA = work.tile([128, 64], f32, tag="gsA")
                            nc.vector.tensor_add(
                                out=gsA, in0=psg,
                                in1=xpb[:, 192:256, tt],
                            )
                            tg = work.tile([128, 64], f32, tag="tg")
                            nc.scalar.activation(out=tg, in_=gsA, func=AF.Tanh)

                            for j, m in enumerate(range(0, 8)):  # i, f
                                for k in range(K_CH):
                                    nc.tensor.matmul(
                                        pif[:, j * 16:(j + 1) * 16],
                                        lhsT=whh_sb[:, k, m * 128:(m + 1) * 128],
                                        rhs=h_sb[:, k * BL:(k + 1) * BL],
                                        start=(k == 0), stop=(k == K_CH - 1),
                                    )
                            sifA = work.tile([128, 128], f32, tag="sifA")
                            nc.vector.tensor_add(
                                out=sifA, in0=pif, in1=xpb[:, 0:128, tt])
                            sif = work.tile([128, 128], f32, tag="sif")
                            nc.scalar.activation(out=sif, in_=sifA, func=AF.Sigmoid)

                            for j, m in enumerate(range(8, 12)):  # o
                                for k in range(K_CH):
                                    nc.tensor.matmul(
                                        pso[:, j * 16:(j + 1) * 16],
                                        lhsT=whh_sb[:, k, m * 128:(m + 1) * 128],
                                        rhs=h_sb[:, k * BL:(k + 1) * BL],
                                        start=(k == 0), stop=(k == K_CH - 1),
                                    )
                            soA = work.tile([128, 64], f32, tag="soA")
                            nc.vector.tensor_add(
                                out=soA, in0=pso, in1=xpb[:, 128:192, tt])
                            so = work.tile([128, 64], f32, tag="so")
                            nc.scalar.activation(out=so, in_=soA, func=AF.Sigmoid)

                            t1 = work.tile([128, 64], f32, tag="t1")
                            nc.vector.tensor_mul(out=t1, in0=sif[:, 0:64], in1=tg)
                            t2 = work.tile([128, 64], f32, tag="t2")
                            nc.vector.tensor_mul(out=t2, in0=sif[:, 64:128], in1=c_sb)
                            nc.vector.tensor_add(out=c_sb, in0=t1, in1=t2)
                            tch = work.tile([128, 64], f32, tag="tch")
                            nc.scalar.activation(out=tch, in_=c_sb, func=AF.Tanh)
                            nc.vector.tensor_mul(out=h_sb, in0=so, in1=tch)

                            pt = work.tile([128, 64], f32, tag="pt")
                            nc.gpsimd.tensor_mul(
                                out=pt, in0=h_sb,
                                in1=_bc_free(sc_sb[:, t, :], K_CH, BL),
                            )
                            nc.gpsimd.tensor_add(out=acc, in0=acc, in1=pt)

                # ---- Phase 3: head ----
                with tc.tile_pool(name="head", bufs=1) as head, \
                     tc.tile_pool(name="head_ps", bufs=1, space="PSUM") as head_ps:
                    nc.sync.dma_start(out=out_pooled[:, :], in_=acc)
                    acch = head.tile([128, K_CH * BL], f16)
                    nc.vector.tensor_copy(out=acch, in_=acc)
                    ps1 = head_ps.tile([128, MO_CH * BL], f32)
                    for mo in range(MO_CH):
                        for k in range(K_CH):
                            nc.tensor.matmul(
                                ps1[:, mo * BL:(mo + 1) * BL],
                                lhsT=w1t_sb[:, k, mo * 128:(mo + 1) * 128],
                                rhs=acch[:, k * BL:(k + 1) * BL],
                                start=(k == 0), stop=(k == K_CH - 1),
                            )
                    p1_sb = head.tile([128, MO_CH * BL], f32)
                    nc.vector.tensor_copy(out=p1_sb, in_=ps1)
                    if use_collective:
                        nc.sync.dma_start(out=ar_in[:, :], in_=p1_sb)
                        nc.gpsimd.collective_compute(
                            "AllReduce",
                            ALU.add,
                            replica_groups=[[0, 1], [2, 3], [4, 5], [6, 7]],
                            ins=[ar_in[:, :].opt()],
                            outs=[ar_out[:, :].opt()],
                        )
                        r_sb = head.tile([128, MO_CH * BL], f32)
                        nc.sync.dma_start(out=r_sb, in_=ar_out[:, :])
                    else:
                        r_sb = p1_sb
                    h1 = head.tile([128, MO_CH * BL], f16)
                    for mo in range(MO_CH):
                        nc.scalar.activation(
                            out=h1[:, mo * BL:(mo + 1) * BL],
                            in_=r_sb[:, mo * BL:(mo + 1) * BL],
                            func=AF.Relu,
                            bias=b1T_sb[:, mo:mo + 1],
                        )
                    ps2 = head_ps.tile([128, MT_CH * BL], f32)
                    for mt in range(MT_CH):
                        for ko in range(MO_CH):
                            nc.tensor.matmul(
                                ps2[:, mt * BL:(mt + 1) * BL],
                                lhsT=w2t_sb[:, ko, mt * 128:(mt + 1) * 128],
                                rhs=h1[:, ko * BL:(ko + 1) * BL],
                                start=(ko == 0), stop=(ko == MO_CH - 1),
                            )
                    lg_sb = head.tile([128, MT_CH * BL], f32)
                    for mt in range(MT_CH):
                        nc.vector.tensor_scalar(
                            out=lg_sb[:, mt * BL:(mt + 1) * BL],
                            in0=ps2[:, mt * BL:(mt + 1) * BL],
                            scalar1=b2T_sb[:, mt:mt + 1], scalar2=None,
                            op0=ALU.add,
                        )
                    nc.sync.dma_start(out=out_logits[:, :], in_=lg_sb)

    nc.compile()
    return nc


def _tile_kxg(w, n_k):
    """[G, K] weight (already permuted rows) -> [n_k, 128, G] fp16 with
    out[k, kk, g] = w[g, k*128+kk]."""
    K = n_k * 128
    wt = w.T.astype(np.float32)  # [K, G]
    return np.ascontiguousarray(
        wt.reshape(n_k, 128, -1)).astype(np.float16)


def prep_core_inputs(x_dir, wih_p, whh_p, bias_p, sc_tb, fc1_w, fc1_b,
                     fc2_w, fc2_b, direction):
    """Build the per-core input map. x_dir [BL, T, E] f32 (already reversed for
    bwd), weights already gate-permuted."""
    ins = {}
    # xt [E_CH, 128, BL*T]: xt[k][kk][b*T+t] = x_dir[b,t,k*128+kk]
    xtt = x_dir.transpose(2, 0, 1).reshape(E_CH, 128, BL * T)
    ins["xt"] = np.ascontiguousarray(xtt).astype(np.float16)
    ins["wih"] = _tile_kxg(wih_p, E_CH)
    ins["whh"] = _tile_kxg(whh_p, K_CH)
    ins["biasT"] = np.ascontiguousarray(
        bias_p.reshape(M_CH, 128).T).astype(np.float32)
    # sc [128, T, BL] replicated over partitions
    ins["sc"] = np.broadcast_to(
        sc_tb.astype(np.float16)[None, :, :], (128, T, BL)).copy()
    w1d = fc1_w[:, direction * H:(direction + 1) * H]  # [OH, H]
    ins["w1t"] = _tile_kxg(w1d, K_CH)
    ins["b1T"] = np.ascontiguousarray(
        fc1_b.reshape(MO_CH, 128).T).astype(np.float32)
    ins["w2t"] = _tile_kxg(fc2_w, MO_CH)
    ins["b2T"] = np.ascontiguousarray(
        fc2_b.reshape(MT_CH, 128).T).astype(np.float32)
    return ins


_NC_CACHE = {}
LAST_RESULT = None


def kernel(x, lengths, attn_w, Wih_f, Whh_f, bih_f, bhh_f,
           Wih_b, Whh_b, bih_b, bhh_b, fc1_w, fc1_b, fc2_w, fc2_b):
    x = np.asarray(x, np.float32)
    lengths = np.asarray(lengths, np.int32)
    attn_w = np.asarray(attn_w, np.float32)
    use_collective = os.environ.get("LSTM_NO_COLLECTIVE", "0") != "1"

    key = (T, use_collective)
    if key not in _NC_CACHE:
        _NC_CACHE[key] = build_nc(T, use_collective)
    nc = _NC_CACHE[key]

    # softmax over attn_w (host glue, exact fp32 as in reference)
    aw = attn_w - attn_w.max()
    e = np.exp(aw)
    scores = (e / e.sum()).astype(np.float32)  # [T]

    tr = np.arange(T)
    # forward sc: sc_f[t, b] = scores[t] * (t < len_b)
    # backward sc: sc_b[tau, b] = scores[len_b-1-tau] * (tau < len_b)
    in_maps = []
    for g in range(4):
        bsl = slice(g * BL, (g + 1) * BL)
        xg = x[bsl]                      # [BL, T, E]
        lg = lengths[bsl]                # [BL]
        mask = tr[:, None] < lg[None, :]  # [T, BL]
        sc_f = scores[:, None] * mask
        idx = np.clip(lg[None, :] - 1 - tr[:, None], 0, T - 1)  # [T, BL]
        sc_b = scores[idx] * mask
        # x reversed per sequence (zeros past length)
        idxc = np.clip(lg[:, None] - 1 - tr[None, :], 0, T - 1)  # [BL, T]
        xrev = np.take_along_axis(xg, idxc[:, :, None], axis=1)
        xrev = xrev * mask.T[:, :, None]

        bias_f = (bih_f + bhh_f)[_GPERM].astype(np.float32)
        bias_b = (bih_b + bhh_b)[_GPERM].astype(np.float32)
        in_maps.append(prep_core_inputs(
            xg, Wih_f[_GPERM], Whh_f[_GPERM], bias_f, sc_f,
            fc1_w, fc1_b, fc2_w, fc2_b, 0))
        in_maps.append(prep_core_inputs(
            xrev, Wih_b[_GPERM], Whh_b[_GPERM], bias_b, sc_b,
            fc1_w, fc1_b, fc2_w, fc2_b, 1))

    trace = os.environ.get("LSTM_TRACE", "0") == "1"
    res = run_bass_kernel_spmd(nc, in_maps, list(range(8)), trace=trace)
    results = res.results
    global LAST_RESULT
    LAST_RESULT = res

    out = np.empty((B, T), np.float32)
    for g in range(4):
        if use_collective:
            lt = results[2 * g]["out_logits"]  # [128, MT_CH*BL]
            lg_out = lt.reshape(128, MT_CH, BL).transpose(2, 1, 0).reshape(BL, T)
        else:
            # host head from pooled partials
            pf = results[2 * g]["out_pooled"]
            pb = results[2 * g + 1]["out_pooled"]
            pooled = np.concatenate(
                [pf.reshape(128, K_CH, BL).transpose(2, 1, 0).reshape(BL, H),
                 pb.reshape(128, K_CH, BL).transpose(2, 1, 0).reshape(BL, H)],
                axis=1)
            h1 = np.maximum(pooled @ fc1_w.T + fc1_b, 0.0)
            lg_out = h1 @ fc2_w.T + fc2_b
        out[g * BL:(g + 1) * BL] = lg_out
    tmask = tr[None, :] < lengths[:, None]
    return np.where(tmask, out, np.float32(-1e30)).astype(np.float32)



# revision 22
# speedup vs baseline: 1.0246x; 1.0246x over previous
"""Trainium2 Bass kernel for bidirectional masked-LSTM + attention pooling + FC head.

Problem (hardcoded shapes): B=64, T=512, E=256, H=512, OH=1024.
  - x [B,T,E] f32, lengths [B] i32, attn_w [T] f32
  - per-direction LSTM weights Wih [4H,E], Whh [4H,H], biases [4H]
  - fc1 [OH,2H]+[OH], fc2 [T,OH]+[T]
  - out: logits [B,T] f32, padded positions = -1e30

Sharding: 8 cores = 4 batch groups (16 seqs) x 2 directions. Each core runs one
direction's full 512-step recurrence for its 16 sequences.

Key structure (v2): no input-projection phase and no DRAM xp spill. x stays in
SBUF; per step the gate pre-activations accumulate in PSUM from three matmul
series that share an accumulation group per 16-col slice:
  bias (rank-1 matmul vs a ones row) -> Wih.x_t (2 k-chunks) -> Whh.h (4 k-chunks)
The bias+Wih matmuls do not depend on h, so they execute during the previous
step's elementwise tail; only the 64 Whh matmuls sit on the serial h->h chain.
Gates use three separate PSUM tiles (g / i,f / o in permuted order [i,f,o,g])
so tanh(g) can start while the i,f,o matmuls still run, and the activations
read PSUM directly. Attention pooling is a masked weighted accumulate on the
Pool engine (per-(t,b) scale table precomputed on host, which also implements
sequence reversal for the backward direction). The FC head runs on every core;
forward/backward pooled partials are combined with a pairwise AllReduce.

Layouts (per core):
  h "hidden-tiled" [128, K_CH*16]: h[b, hid] at partition hid%128, col (hid//128)*16+b.
  PSUM gate tiles: col j*16+b holds gate chunk m (m=j+offset), partition = gate%128.
"""

import os

import numpy as np

import concourse.bass as bass
import concourse.tile as tile
from concourse import bacc, mybir
from concourse.bass_utils import run_bass_kernel_spmd

B, T, E, H, OH = 64, 512, 256, 512, 1024
G = 4 * H          # 2048 gates
BL = 16            # batch per core
M_CH = G // 128    # 16 gate chunks
K_CH = H // 128    # 4 hidden chunks
E_CH = E // 128    # 2 input chunks
MO_CH = OH // 128  # 8
MT_CH = T // 128   # 4

f32 = mybir.dt.float32
f16 = mybir.dt.float16
f8 = mybir.dt.float8e4
AF = mybir.ActivationFunctionType
ALU = mybir.AluOpType
DR = mybir.MatmulPerfMode.DoubleRow

# gate permutation: torch order [i,f,g,o] -> kernel order [i,f,o,g]
# perm[new_pos] = old_index  (applied to rows of Wih/Whh and bias)
_GPERM = np.concatenate([
    np.arange(0, H),          # i
    np.arange(H, 2 * H),      # f
    np.arange(3 * H, 4 * H),  # o
    np.arange(2 * H, 3 * H),  # g
])

# (psum tag, m-chunk list) per gate group; emission order of the Whh series is
# g first so tanh(g) overlaps the remaining matmuls.
_GROUPS = [
    ("Pg", list(range(12, 16))),   # g
    ("Pif", list(range(0, 8))),    # i, f
    ("Po", list(range(8, 12))),    # o
]


def _bc_free(ap, reps, width):
    """AP that broadcasts a [P, width] slice to [P, reps, width] via stride-0."""
    return bass.AP(
        tensor=ap.tensor,
        offset=ap.offset,
        ap=[ap.ap[0], [0, reps]] + list(ap.ap[1:]),
    )


def build_nc(t_steps=T, use_collective=True, fp8=False):
    nc = bacc.Bacc("TRN2", target_bir_lowering=False, num_devices=8)

    # ---- DRAM parameters (per-core payloads prepared on host) ----
    xt = nc.declare_dram_parameter("xt", [E_CH, 128, T * BL], f16, isOutput=False)
    wih = nc.declare_dram_parameter("wih", [E_CH, 128, G], f16, isOutput=False)
    if fp8:
        # DoubleRow layout: whh8[cc][p][i][g] = Whh[g, cc*256 + i*128 + p]
        whh8 = nc.declare_dram_parameter("whh8", [K_CH // 2, 128, 2, G], f8,
                                         isOutput=False)
    else:
        whh = nc.declare_dram_parameter("whh", [K_CH, 128, G], f16,
                                        isOutput=False)
    bias_row = nc.declare_dram_parameter("bias_row", [128, G], f16, isOutput=False)
    ones16 = nc.declare_dram_parameter("ones16", [128, BL], f16, isOutput=False)
    sc = nc.declare_dram_parameter("sc", [128, T, BL], f16, isOutput=False)
    w1t = nc.declare_dram_parameter("w1t", [K_CH, 128, OH], f16, isOutput=False)
    b1T = nc.declare_dram_parameter("b1T", [128, MO_CH], f32, isOutput=False)
    w2t = nc.declare_dram_parameter("w2t", [MO_CH, 128, T], f16, isOutput=False)
    b2T = nc.declare_dram_parameter("b2T", [128, MT_CH], f32, isOutput=False)

    out_logits = nc.declare_dram_parameter("out_logits", [128, MT_CH * BL], f32,
                                           isOutput=True)
    out_pooled = nc.declare_dram_parameter("out_pooled", [128, K_CH * BL], f32,
                                           isOutput=True)
    out_h = nc.declare_dram_parameter("out_h", [128, K_CH * BL], f32,
                                      isOutput=True)
    out_c = nc.declare_dram_parameter("out_c", [128, K_CH * BL], f32,
                                      isOutput=True)

    ar_in = nc.dram_tensor("ar_in", [128, MO_CH * BL], f32)
    ar_out = nc.dram_tensor("ar_out", [128, MO_CH * BL], f32)

    with tile.TileContext(nc) as tc:
        with tc.tile_pool(name="const", bufs=1) as const_pool:
            if fp8:
                whh8_sb = const_pool.tile([128, K_CH // 2, 2, G], f8)
                for cc in range(K_CH // 2):
                    nc.sync.dma_start(out=whh8_sb[:, cc, :, :], in_=whh8[cc])
            else:
                whh_sb = const_pool.tile([128, K_CH, G], f16)
                for k in range(K_CH):
                    nc.sync.dma_start(out=whh_sb[:, k, :], in_=whh[k])
            wih_sb = const_pool.tile([128, E_CH, G], f16)
            for k in range(E_CH):
                nc.sync.dma_start(out=wih_sb[:, k, :], in_=wih[k])
            xt_sb = const_pool.tile([128, E_CH, T * BL], f16)
            for k in range(E_CH):
                nc.sync.dma_start(out=xt_sb[:, k, :], in_=xt[k])
            biask_sb = const_pool.tile([128, M_CH, 128], f16)
            nc.sync.dma_start(out=biask_sb[:, :, :], in_=bias_row[:, :])
            ones_sb = const_pool.tile([128, BL], f16)
            nc.sync.dma_start(out=ones_sb, in_=ones16[:, :])
            sc_sb = const_pool.tile([128, T, BL], f16)
            nc.sync.dma_start(out=sc_sb, in_=sc[:, :, :])
            w1t_sb = const_pool.tile([128, K_CH, OH], f16)
            for k in range(K_CH):
                nc.sync.dma_start(out=w1t_sb[:, k, :], in_=w1t[k])
            b1T_sb = const_pool.tile([128, MO_CH], f32)
            nc.sync.dma_start(out=b1T_sb, in_=b1T[:, :])
            w2t_sb = const_pool.tile([128, MO_CH, T], f16)
            for k in range(MO_CH):
                nc.sync.dma_start(out=w2t_sb[:, k, :], in_=w2t[k])
            b2T_sb = const_pool.tile([128, MT_CH], f32)
            nc.sync.dma_start(out=b2T_sb, in_=b2T[:, :])

            # ---- recurrence ----
            with tc.tile_pool(name="state", bufs=1) as state_pool:
                h_sb = state_pool.tile([128, K_CH * BL], f16)
                c_sb = state_pool.tile([128, K_CH * BL], f32)
                acc = state_pool.tile([128, K_CH * BL], f32)
                nc.vector.memset(h_sb, 0.0)
                nc.vector.memset(c_sb, 0.0)
                nc.gpsimd.memset(acc, 0.0)
                if fp8:
                    # fp8 copy of h for the DoubleRow Whh matmul; col order
                    # (k,b) with k = cc*2+i matches the (cc,i,b) nesting.
                    h8_sb = state_pool.tile([128, K_CH // 2, 2, BL], f8)
                    nc.vector.memset(h8_sb, 0.0)

                with tc.tile_pool(name="rec_ps", bufs=2, space="PSUM") as rec_ps, \
                     tc.tile_pool(name="work", bufs=2) as work:
                    for t in range(t_steps):
                        # X tile: bias + Wih.x_t, complete per-slice groups,
                        # one PSUM bank for all 16 slices (regions g|if|o).
                        # No h dependency -> these execute during the previous
                        # step's tail. PSUM accumulation groups must be
                        # contiguous in the PE stream, hence separate X/H
                        # tiles combined by a vector add in the tail.
                        xfull = rec_ps.tile([128, M_CH * BL], f32, tag="X")
                        xtiles = {}
                        off = 0
                        for tag, ms in _GROUPS:
                            xt_ = xfull[:, off * BL:(off + len(ms)) * BL]
                            off += len(ms)
                            xtiles[tag] = xt_
                            for j, m in enumerate(ms):
                                sl = xt_[:, j * BL:(j + 1) * BL]
                                nc.tensor.matmul(
                                    sl, lhsT=biask_sb[:, m, :], rhs=ones_sb,
                                    start=True, stop=False)
                                for k in range(E_CH):
                                    nc.tensor.matmul(
                                        sl,
                                        lhsT=wih_sb[:, k, m * 128:(m + 1) * 128],
                                        rhs=xt_sb[:, k, t * BL:(t + 1) * BL],
                                        start=False, stop=(k == E_CH - 1))
                        # DVE can read only one PSUM input per op: evacuate X
                        # to SBUF (off the critical path, Act has slack).
                        xe = work.tile([128, M_CH * BL], f16, tag="xe")
                        nc.scalar.activation(out=xe, in_=xfull, func=AF.Identity)
                        xtiles = {"Pg": xe[:, 0:4 * BL],
                                  "Pif": xe[:, 4 * BL:12 * BL],
                                  "Po": xe[:, 12 * BL:16 * BL]}
                        # H tiles: Whh.h, per-slice groups; g chunks first so
                        # tanh(g) overlaps the i,f,o matmuls.
                        htiles = {}
                        for tag, ms in _GROUPS:
                            ht_ = rec_ps.tile([128, len(ms) * BL], f32,
                                              tag="H" + tag)
                            htiles[tag] = ht_
                            for j, m in enumerate(ms):
                                sl = ht_[:, j * BL:(j + 1) * BL]
                                if fp8:
                                    for cc in range(K_CH // 2):
                                        nc.tensor.matmul(
                                            sl,
                                            lhsT=whh8_sb[:, cc, :,
                                                         m * 128:(m + 1) * 128],
                                            rhs=h8_sb[:, cc, :, :],
                                            start=(cc == 0),
                                            stop=(cc == K_CH // 2 - 1),
                                            perf_mode=DR,
                                        )
                                else:
                                    for k in range(K_CH):
                                        nc.tensor.matmul(
                                            sl,
                                            lhsT=whh_sb[:, k,
                                                        m * 128:(m + 1) * 128],
                                            rhs=h_sb[:, k * BL:(k + 1) * BL],
                                            start=(k == 0),
                                            stop=(k == K_CH - 1),
                                        )
                        gA = work.tile([128, 4 * BL], f32, tag="gA")
                        nc.vector.tensor_add(out=gA, in0=xtiles["Pg"],
                                             in1=htiles["Pg"])
                        tg = work.tile([128, 4 * BL], f32, tag="tg")
                        nc.scalar.activation(out=tg, in_=gA, func=AF.Tanh)
                        ifA = work.tile([128, 8 * BL], f32, tag="ifA")
                        nc.vector.tensor_add(out=ifA, in0=xtiles["Pif"],
                                             in1=htiles["Pif"])
                        sif = work.tile([128, 8 * BL], f32, tag="sif")
                        nc.scalar.activation(out=sif, in_=ifA, func=AF.Sigmoid)
                        oA = work.tile([128, 4 * BL], f32, tag="oA")
                        nc.vector.tensor_add(out=oA, in0=xtiles["Po"],
                                             in1=htiles["Po"])
                        so = work.tile([128, 4 * BL], f32, tag="so")
                        nc.scalar.activation(out=so, in_=oA, func=AF.Sigmoid)
                        t1 = work.tile([128, 4 * BL], f32, tag="t1")
                        nc.vector.tensor_mul(out=t1, in0=sif[:, 0:4 * BL], in1=tg)
                        t2 = work.tile([128, 4 * BL], f32, tag="t2")
                        nc.vector.tensor_mul(out=t2, in0=sif[:, 4 * BL:8 * BL],
                                             in1=c_sb)
                        nc.vector.tensor_add(out=c_sb, in0=t1, in1=t2)
                        tch = work.tile([128, 4 * BL], f32, tag="tch")
                        nc.scalar.activation(out=tch, in_=c_sb, func=AF.Tanh)
                        if fp8:
                            nc.vector.tensor_mul(out=h8_sb, in0=so, in1=tch)
                        nc.vector.tensor_mul(out=h_sb, in0=so, in1=tch)

                        pt = work.tile([128, 4 * BL], f32, tag="pt")
                        nc.gpsimd.tensor_mul(
                            out=pt, in0=h_sb,
                            in1=_bc_free(sc_sb[:, t, :], K_CH, BL),
                        )
                        nc.gpsimd.tensor_add(out=acc, in0=acc, in1=pt)

                # ---- head ----
                with tc.tile_pool(name="head", bufs=1) as head, \
                     tc.tile_pool(name="head_ps", bufs=1, space="PSUM") as head_ps:
                    nc.sync.dma_start(out=out_pooled[:, :], in_=acc)
                    hf = head.tile([128, K_CH * BL], f32)
                    nc.vector.tensor_copy(out=hf, in_=h_sb)
                    nc.sync.dma_start(out=out_h[:, :], in_=hf)
                    nc.sync.dma_start(out=out_c[:, :], in_=c_sb)
                    acch = head.tile([128, K_CH * BL], f16)
                    nc.vector.tensor_copy(out=acch, in_=acc)
                    ps1 = head_ps.tile([128, MO_CH * BL], f32)
                    for mo in range(MO_CH):
                        for k in range(K_CH):
                            nc.tensor.matmul(
                                ps1[:, mo * BL:(mo + 1) * BL],
                                lhsT=w1t_sb[:, k, mo * 128:(mo + 1) * 128],
                                rhs=acch[:, k * BL:(k + 1) * BL],
                                start=(k == 0), stop=(k == K_CH - 1),
                            )
                    p1_sb = head.tile([128, MO_CH * BL], f32)
                    nc.vector.tensor_copy(out=p1_sb, in_=ps1)
                    if use_collective:
                        nc.sync.dma_start(out=ar_in[:, :], in_=p1_sb)
                        nc.gpsimd.collective_compute(
                            "AllReduce",
                            ALU.add,
                            replica_groups=[[0, 1], [2, 3], [4, 5], [6, 7]],
                            ins=[ar_in[:, :].opt()],
                            outs=[ar_out[:, :].opt()],
                        )
                        r_sb = head.tile([128, MO_CH * BL], f32)
                        nc.sync.dma_start(out=r_sb, in_=ar_out[:, :])
                    else:
                        r_sb = p1_sb
                    h1 = head.tile([128, MO_CH * BL], f16)
                    for mo in range(MO_CH):
                        nc.scalar.activation(
                            out=h1[:, mo * BL:(mo + 1) * BL],
                            in_=r_sb[:, mo * BL:(mo + 1) * BL],
                            func=AF.Relu,
                            bias=b1T_sb[:, mo:mo + 1],
                        )
                    ps2 = head_ps.tile([128, MT_CH * BL], f32)
                    for mt in range(MT_CH):
                        for ko in range(MO_CH):
                            nc.tensor.matmul(
                                ps2[:, mt * BL:(mt + 1) * BL],
                                lhsT=w2t_sb[:, ko, mt * 128:(mt + 1) * 128],
                                rhs=h1[:, ko * BL:(ko + 1) * BL],
                                start=(ko == 0), stop=(ko == MO_CH - 1),
                            )
                    lg_sb = head.tile([128, MT_CH * BL], f32)
                    for mt in range(MT_CH):
                        nc.vector.tensor_scalar(
                            out=lg_sb[:, mt * BL:(mt + 1) * BL],
                            in0=ps2[:, mt * BL:(mt + 1) * BL],
                            scalar1=b2T_sb[:, mt:mt + 1], scalar2=None,
                            op0=ALU.add,
                        )
                    nc.sync.dma_start(out=out_logits[:, :], in_=lg_sb)

    nc.compile()
    return nc


def _tile_kxg(w, n_k):
    """[G, K] weight (already permuted rows) -> [n_k, 128, G] fp16 with
    out[k, kk, g] = w[g, k*128+kk]."""
    wt = w.T.astype(np.float32)  # [K, G]
    return np.ascontiguousarray(
        wt.reshape(n_k, 128, -1)).astype(np.float16)


def prep_core_inputs(x_dir, wih_p, whh_p, bias_p, sc_tb, fc1_w, fc1_b,
                     fc2_w, fc2_b, direction):
    """Build the per-core input map. x_dir [BL, T, E] f32 (already reversed for
    bwd), weights already gate-permuted."""
    import ml_dtypes

    fp8 = os.environ.get("LSTM_FP8", "0") == "1"
    ins = {}
    # xt [E_CH, 128, T*BL]: xt[k][kk][t*BL+b] = x_dir[b,t,k*128+kk]
    xtt = x_dir.transpose(2, 1, 0).reshape(E_CH, 128, T * BL)
    ins["xt"] = np.ascontiguousarray(xtt).astype(np.float16)
    ins["wih"] = _tile_kxg(wih_p, E_CH)
    if fp8:
        wt = whh_p.T.astype(np.float32)  # [K, G]
        w8 = wt.reshape(K_CH // 2, 2, 128, G).transpose(0, 2, 1, 3)
        ins["whh8"] = np.ascontiguousarray(w8).astype(ml_dtypes.float8_e4m3)
    else:
        ins["whh"] = _tile_kxg(whh_p, K_CH)
    br = np.zeros((128, G), np.float16)
    br[0, :] = bias_p.astype(np.float16)
    ins["bias_row"] = br
    on = np.zeros((128, BL), np.float16)
    on[0, :] = 1.0
    ins["ones16"] = on
    # sc [128, T, BL] replicated over partitions
    ins["sc"] = np.broadcast_to(
        sc_tb.astype(np.float16)[None, :, :], (128, T, BL)).copy()
    w1d = fc1_w[:, direction * H:(direction + 1) * H]  # [OH, H]
    ins["w1t"] = _tile_kxg(w1d, K_CH)
    ins["b1T"] = np.ascontiguousarray(
        fc1_b.reshape(MO_CH, 128).T).astype(np.float32)
    ins["w2t"] = _tile_kxg(fc2_w, MO_CH)
    ins["b2T"] = np.ascontiguousarray(
        fc2_b.reshape(MT_CH, 128).T).astype(np.float32)
    return ins


_NC_CACHE = {}
LAST_RESULT = None


def make_in_maps(x, lengths, attn_w, Wih_f, Whh_f, bih_f, bhh_f,
                 Wih_b, Whh_b, bih_b, bhh_b, fc1_w, fc1_b, fc2_w, fc2_b):
    # softmax over attn_w (host glue, exact fp32 as in reference)
    aw = attn_w - attn_w.max()
    e = np.exp(aw)
    scores = (e / e.sum()).astype(np.float32)  # [T]

    tr = np.arange(T)
    # forward sc: sc_f[t, b] = scores[t] * (t < len_b)
    # backward sc: sc_b[tau, b] = scores[len_b-1-tau] * (tau < len_b)
    in_maps = []
    for g in range(4):
        bsl = slice(g * BL, (g + 1) * BL)
        xg = x[bsl]                      # [BL, T, E]
        lg = lengths[bsl]                # [BL]
        mask = tr[:, None] < lg[None, :]  # [T, BL]
        sc_f = scores[:, None] * mask
        idx = np.clip(lg[None, :] - 1 - tr[:, None], 0, T - 1)  # [T, BL]
        sc_b = scores[idx] * mask
        # x reversed per sequence (zeros past length)
        idxc = np.clip(lg[:, None] - 1 - tr[None, :], 0, T - 1)  # [BL, T]
        xrev = np.take_along_axis(xg, idxc[:, :, None], axis=1)
        xrev = xrev * mask.T[:, :, None]

        bias_f = (bih_f + bhh_f)[_GPERM].astype(np.float32)
        bias_b = (bih_b + bhh_b)[_GPERM].astype(np.float32)
        in_maps.append(prep_core_inputs(
            xg, Wih_f[_GPERM], Whh_f[_GPERM], bias_f, sc_f,
            fc1_w, fc1_b, fc2_w, fc2_b, 0))
        in_maps.append(prep_core_inputs(
            xrev, Wih_b[_GPERM], Whh_b[_GPERM], bias_b, sc_b,
            fc1_w, fc1_b, fc2_w, fc2_b, 1))
    return in_maps


def kernel(x, lengths, attn_w, Wih_f, Whh_f, bih_f, bhh_f,
           Wih_b, Whh_b, bih_b, bhh_b, fc1_w, fc1_b, fc2_w, fc2_b):
    x = np.asarray(x, np.float32)
    lengths = np.asarray(lengths, np.int32)
    attn_w = np.asarray(attn_w, np.float32)
    use_collective = os.environ.get("LSTM_NO_COLLECTIVE", "0") != "1"
    fp8 = os.environ.get("LSTM_FP8", "0") == "1"

    key = (T, use_collective, fp8)
    if key not in _NC_CACHE:
        _NC_CACHE[key] = build_nc(T, use_collective, fp8=fp8)
    nc = _NC_CACHE[key]

    in_maps = make_in_maps(x, lengths, attn_w, Wih_f, Whh_f, bih_f, bhh_f,
                           Wih_b, Whh_b, bih_b, bhh_b, fc1_w, fc1_b,
                           fc2_w, fc2_b)
    tr = np.arange(T)
    trace = os.environ.get("LSTM_TRACE", "0") == "1"
    res = run_bass_kernel_spmd(nc, in_maps, list(range(8)), trace=trace)
    results = res.results
    global LAST_RESULT
    LAST_RESULT = res

    out = np.empty((B, T), np.float32)
    for g in range(4):
        if use_collective:
            lt = results[2 * g]["out_logits"]  # [128, MT_CH*BL]
            lg_out = lt.reshape(128, MT_CH, BL).transpose(2, 1, 0).reshape(BL, T)
        else:
            # host head from pooled partials
            pf = results[2 * g]["out_pooled"]
            pb = results[2 * g + 1]["out_pooled"]
            pooled = np.concatenate(
                [pf.reshape(128, K_CH, BL).transpose(2, 1, 0).reshape(BL, H),
                 pb.reshape(128, K_CH, BL).transpose(2, 1, 0).reshape(BL, H)],
                axis=1)
            h1 = np.maximum(pooled @ fc1_w.T + fc1_b, 0.0)
            lg_out = h1 @ fc2_w.T + fc2_b
        out[g * BL:(g + 1) * BL] = lg_out
    tmask = tr[None, :] < lengths[:, None]
    return np.where(tmask, out, np.float32(-1e30)).astype(np.float32)


# revision 25
# speedup vs baseline: 1.5180x; 1.4816x over previous
"""Trainium2 Bass kernel for bidirectional masked-LSTM + attention pooling + FC head.

Problem (hardcoded shapes): B=64, T=512, E=256, H=512, OH=1024.
  - x [B,T,E] f32, lengths [B] i32, attn_w [T] f32
  - per-direction LSTM weights Wih [4H,E], Whh [4H,H], biases [4H]
  - fc1 [OH,2H]+[OH], fc2 [T,OH]+[T]
  - out: logits [B,T] f32, padded positions = -1e30

Sharding: 8 cores = 4 batch groups (16 seqs) x 2 directions. Each core runs one
direction's full 512-step recurrence for its 16 sequences.

Key structure (v2): no input-projection phase and no DRAM xp spill. x stays in
SBUF; per step the gate pre-activations accumulate in PSUM from three matmul
series that share an accumulation group per 16-col slice:
  bias (rank-1 matmul vs a ones row) -> Wih.x_t (2 k-chunks) -> Whh.h (4 k-chunks)
The bias+Wih matmuls do not depend on h, so they execute during the previous
step's elementwise tail; only the 64 Whh matmuls sit on the serial h->h chain.
Gates use three separate PSUM tiles (g / i,f / o in permuted order [i,f,o,g])
so tanh(g) can start while the i,f,o matmuls still run, and the activations
read PSUM directly. Attention pooling is a masked weighted accumulate on the
Pool engine (per-(t,b) scale table precomputed on host, which also implements
sequence reversal for the backward direction). The FC head runs on every core;
forward/backward pooled partials are combined with a pairwise AllReduce.

Layouts (per core):
  h "hidden-tiled" [128, K_CH*16]: h[b, hid] at partition hid%128, col (hid//128)*16+b.
  PSUM gate tiles: col j*16+b holds gate chunk m (m=j+offset), partition = gate%128.
"""

import os

import numpy as np

import concourse.bass as bass
import concourse.tile as tile
from concourse import bacc, mybir
from concourse.bass_utils import run_bass_kernel_spmd

B, T, E, H, OH = 64, 512, 256, 512, 1024
G = 4 * H          # 2048 gates
BL = 16            # batch per core
M_CH = G // 128    # 16 gate chunks
K_CH = H // 128    # 4 hidden chunks
E_CH = E // 128    # 2 input chunks
MO_CH = OH // 128  # 8
MT_CH = T // 128   # 4

f32 = mybir.dt.float32
f16 = mybir.dt.float16
f8 = mybir.dt.float8e4
AF = mybir.ActivationFunctionType
ALU = mybir.AluOpType
DR = mybir.MatmulPerfMode.DoubleRow

# gate permutation: torch order [i,f,g,o] -> kernel order [i,f,o,g]
# perm[new_pos] = old_index  (applied to rows of Wih/Whh and bias)
_GPERM = np.concatenate([
    np.arange(0, H),          # i
    np.arange(H, 2 * H),      # f
    np.arange(3 * H, 4 * H),  # o
    np.arange(2 * H, 3 * H),  # g
])

# (psum tag, m-chunk list) per gate group; emission order of the Whh series is
# g first so tanh(g) overlaps the remaining matmuls.
_GROUPS = [
    ("Pg", list(range(12, 16))),   # g
    ("Pif", list(range(0, 8))),    # i, f
    ("Po", list(range(8, 12))),    # o
]


def _bc_free(ap, reps, width):
    """AP that broadcasts a [P, width] slice to [P, reps, width] via stride-0."""
    return bass.AP(
        tensor=ap.tensor,
        offset=ap.offset,
        ap=[ap.ap[0], [0, reps]] + list(ap.ap[1:]),
    )


def build_nc(t_steps=T, use_collective=True, fp8=False):
    nc = bacc.Bacc("TRN2", target_bir_lowering=False, num_devices=8)

    # ---- DRAM parameters (per-core payloads prepared on host) ----
    xt = nc.declare_dram_parameter("xt", [E_CH, 128, T * BL], f16, isOutput=False)
    wih = nc.declare_dram_parameter("wih", [E_CH, 128, G], f16, isOutput=False)
    if fp8:
        # DoubleRow layout: whh8[cc][p][i][g] = Whh[g, cc*256 + i*128 + p]
        whh8 = nc.declare_dram_parameter("whh8", [K_CH // 2, 128, 2, G], f8,
                                         isOutput=False)
    else:
        whh = nc.declare_dram_parameter("whh", [K_CH, 128, G], f16,
                                        isOutput=False)
    bias_row = nc.declare_dram_parameter("bias_row", [128, G], f16, isOutput=False)
    ones16 = nc.declare_dram_parameter("ones16", [128, BL], f16, isOutput=False)
    sc = nc.declare_dram_parameter("sc", [128, T, BL], f16, isOutput=False)
    w1t = nc.declare_dram_parameter("w1t", [K_CH, 128, OH], f16, isOutput=False)
    b1T = nc.declare_dram_parameter("b1T", [128, MO_CH], f32, isOutput=False)
    w2t = nc.declare_dram_parameter("w2t", [MO_CH, 128, T], f16, isOutput=False)
    b2T = nc.declare_dram_parameter("b2T", [128, MT_CH], f32, isOutput=False)

    out_logits = nc.declare_dram_parameter("out_logits", [128, MT_CH * BL], f32,
                                           isOutput=True)
    out_pooled = nc.declare_dram_parameter("out_pooled", [128, K_CH * BL], f32,
                                           isOutput=True)
    out_h = nc.declare_dram_parameter("out_h", [128, K_CH * BL], f32,
                                      isOutput=True)
    out_c = nc.declare_dram_parameter("out_c", [128, K_CH * BL], f32,
                                      isOutput=True)

    ar_in = nc.dram_tensor("ar_in", [128, MO_CH * BL], f32)
    ar_out = nc.dram_tensor("ar_out", [128, MO_CH * BL], f32)

    with tile.TileContext(nc) as tc:
        with tc.tile_pool(name="const", bufs=1) as const_pool:
            if fp8:
                whh8_sb = const_pool.tile([128, K_CH // 2, 2, G], f8)
                for cc in range(K_CH // 2):
                    nc.sync.dma_start(out=whh8_sb[:, cc, :, :], in_=whh8[cc])
            else:
                whh_sb = const_pool.tile([128, K_CH, G], f16)
                for k in range(K_CH):
                    nc.sync.dma_start(out=whh_sb[:, k, :], in_=whh[k])
            wih_sb = const_pool.tile([128, E_CH, G], f16)
            for k in range(E_CH):
                nc.sync.dma_start(out=wih_sb[:, k, :], in_=wih[k])
            xt_sb = const_pool.tile([128, E_CH, T * BL], f16)
            for k in range(E_CH):
                nc.sync.dma_start(out=xt_sb[:, k, :], in_=xt[k])
            biask_sb = const_pool.tile([128, M_CH, 128], f16)
            nc.sync.dma_start(out=biask_sb[:, :, :], in_=bias_row[:, :])
            ones_sb = const_pool.tile([128, BL], f16)
            nc.sync.dma_start(out=ones_sb, in_=ones16[:, :])
            sc_sb = const_pool.tile([128, T, BL], f16)
            nc.sync.dma_start(out=sc_sb, in_=sc[:, :, :])
            w1t_sb = const_pool.tile([128, K_CH, OH], f16)
            for k in range(K_CH):
                nc.sync.dma_start(out=w1t_sb[:, k, :], in_=w1t[k])
            b1T_sb = const_pool.tile([128, MO_CH], f32)
            nc.sync.dma_start(out=b1T_sb, in_=b1T[:, :])
            w2t_sb = const_pool.tile([128, MO_CH, T], f16)
            for k in range(MO_CH):
                nc.sync.dma_start(out=w2t_sb[:, k, :], in_=w2t[k])
            b2T_sb = const_pool.tile([128, MT_CH], f32)
            nc.sync.dma_start(out=b2T_sb, in_=b2T[:, :])

            # ---- recurrence ----
            with tc.tile_pool(name="state", bufs=1) as state_pool:
                h_sb = state_pool.tile([128, K_CH * BL], f16)
                c_sb = state_pool.tile([128, K_CH * BL], f32)
                acc = state_pool.tile([128, K_CH * BL], f32)
                nc.vector.memset(h_sb, 0.0)
                nc.vector.memset(c_sb, 0.0)
                nc.gpsimd.memset(acc, 0.0)
                if fp8:
                    # fp8 copy of h for the DoubleRow Whh matmul; col order
                    # (k,b) with k = cc*2+i matches the (cc,i,b) nesting.
                    h8_sb = state_pool.tile([128, K_CH // 2, 2, BL], f8)
                    nc.vector.memset(h8_sb, 0.0)

                with tc.tile_pool(name="rec_ps", bufs=2, space="PSUM") as rec_ps, \
                     tc.tile_pool(name="work", bufs=2) as work:
                    from contextlib import ExitStack
                    for t in range(t_steps):
                        # Gate each step's instructions at virtual time t ms:
                        # the compile-time list scheduler otherwise hoists
                        # future X-phases ahead of the h-critical Whh
                        # matmuls (its cost model over-estimates the step,
                        # which becomes self-fulfilling in the static order).
                        step_gate = ExitStack()
                        step_gate.enter_context(tc.tile_wait_until(ms=t))
                        # X tile: bias + Wih.x_t, complete per-slice groups,
                        # one PSUM bank for all 16 slices (regions g|if|o).
                        # No h dependency -> these execute during the previous
                        # step's tail. PSUM accumulation groups must be
                        # contiguous in the PE stream, hence separate X/H
                        # tiles combined by a vector add in the tail.
                        xfull = rec_ps.tile([128, M_CH * BL], f32, tag="X")
                        xtiles = {}
                        off = 0
                        for tag, ms in _GROUPS:
                            xt_ = xfull[:, off * BL:(off + len(ms)) * BL]
                            off += len(ms)
                            xtiles[tag] = xt_
                            for j, m in enumerate(ms):
                                sl = xt_[:, j * BL:(j + 1) * BL]
                                nc.tensor.matmul(
                                    sl, lhsT=biask_sb[:, m, :], rhs=ones_sb,
                                    start=True, stop=False)
                                for k in range(E_CH):
                                    nc.tensor.matmul(
                                        sl,
                                        lhsT=wih_sb[:, k, m * 128:(m + 1) * 128],
                                        rhs=xt_sb[:, k, t * BL:(t + 1) * BL],
                                        start=False, stop=(k == E_CH - 1))
                        # DVE can read only one PSUM input per op: evacuate X
                        # to SBUF (off the critical path, Act has slack).
                        xe = work.tile([128, M_CH * BL], f16, tag="xe")
                        nc.scalar.activation(out=xe, in_=xfull, func=AF.Identity)
                        xtiles = {"Pg": xe[:, 0:4 * BL],
                                  "Pif": xe[:, 4 * BL:12 * BL],
                                  "Po": xe[:, 12 * BL:16 * BL]}
                        # H tiles: Whh.h, per-slice groups; g chunks first so
                        # tanh(g) overlaps the i,f,o matmuls.
                        htiles = {}
                        for tag, ms in _GROUPS:
                            ht_ = rec_ps.tile([128, len(ms) * BL], f32,
                                              tag="H" + tag)
                            htiles[tag] = ht_
                            for j, m in enumerate(ms):
                                sl = ht_[:, j * BL:(j + 1) * BL]
                                if fp8:
                                    for cc in range(K_CH // 2):
                                        nc.tensor.matmul(
                                            sl,
                                            lhsT=whh8_sb[:, cc, :,
                                                         m * 128:(m + 1) * 128],
                                            rhs=h8_sb[:, cc, :, :],
                                            start=(cc == 0),
                                            stop=(cc == K_CH // 2 - 1),
                                            perf_mode=DR,
                                        )
                                else:
                                    for k in range(K_CH):
                                        nc.tensor.matmul(
                                            sl,
                                            lhsT=whh_sb[:, k,
                                                        m * 128:(m + 1) * 128],
                                            rhs=h_sb[:, k * BL:(k + 1) * BL],
                                            start=(k == 0),
                                            stop=(k == K_CH - 1),
                                        )
                        gA = work.tile([128, 4 * BL], f16, tag="gA")
                        nc.vector.tensor_add(out=gA, in0=xtiles["Pg"],
                                             in1=htiles["Pg"])
                        tg = work.tile([128, 4 * BL], f16, tag="tg")
                        nc.scalar.activation(out=tg, in_=gA, func=AF.Tanh)
                        ifA = work.tile([128, 8 * BL], f16, tag="ifA")
                        nc.vector.tensor_add(out=ifA, in0=xtiles["Pif"],
                                             in1=htiles["Pif"])
                        sif = work.tile([128, 8 * BL], f16, tag="sif")
                        nc.scalar.activation(out=sif, in_=ifA, func=AF.Sigmoid)
                        oA = work.tile([128, 4 * BL], f16, tag="oA")
                        nc.vector.tensor_add(out=oA, in0=xtiles["Po"],
                                             in1=htiles["Po"])
                        so = work.tile([128, 4 * BL], f16, tag="so")
                        nc.scalar.activation(out=so, in_=oA, func=AF.Sigmoid)
                        t1 = work.tile([128, 4 * BL], f16, tag="t1")
                        nc.vector.tensor_mul(out=t1, in0=sif[:, 0:4 * BL], in1=tg)
                        t2 = work.tile([128, 4 * BL], f32, tag="t2")
                        nc.vector.tensor_mul(out=t2, in0=sif[:, 4 * BL:8 * BL],
                                             in1=c_sb)
                        nc.vector.tensor_add(out=c_sb, in0=t1, in1=t2)
                        tch = work.tile([128, 4 * BL], f16, tag="tch")
                        nc.scalar.activation(out=tch, in_=c_sb, func=AF.Tanh)
                        if fp8:
                            nc.vector.tensor_mul(out=h8_sb, in0=so, in1=tch)
                        nc.vector.tensor_mul(out=h_sb, in0=so, in1=tch)

                        pt = work.tile([128, 4 * BL], f32, tag="pt")
                        nc.gpsimd.tensor_mul(
                            out=pt, in0=h_sb,
                            in1=_bc_free(sc_sb[:, t, :], K_CH, BL),
                        )
                        nc.gpsimd.tensor_add(out=acc, in0=acc, in1=pt)
                        step_gate.close()

                # ---- head ----
                with tc.tile_pool(name="head", bufs=1) as head, \
                     tc.tile_pool(name="head_ps", bufs=1, space="PSUM") as head_ps:
                    nc.sync.dma_start(out=out_pooled[:, :], in_=acc)
                    hf = head.tile([128, K_CH * BL], f32)
                    nc.vector.tensor_copy(out=hf, in_=h_sb)
                    nc.sync.dma_start(out=out_h[:, :], in_=hf)
                    nc.sync.dma_start(out=out_c[:, :], in_=c_sb)
                    acch = head.tile([128, K_CH * BL], f16)
                    nc.vector.tensor_copy(out=acch, in_=acc)
                    ps1 = head_ps.tile([128, MO_CH * BL], f32)
                    for mo in range(MO_CH):
                        for k in range(K_CH):
                            nc.tensor.matmul(
                                ps1[:, mo * BL:(mo + 1) * BL],
                                lhsT=w1t_sb[:, k, mo * 128:(mo + 1) * 128],
                                rhs=acch[:, k * BL:(k + 1) * BL],
                                start=(k == 0), stop=(k == K_CH - 1),
                            )
                    p1_sb = head.tile([128, MO_CH * BL], f32)
                    nc.vector.tensor_copy(out=p1_sb, in_=ps1)
                    if use_collective:
                        nc.sync.dma_start(out=ar_in[:, :], in_=p1_sb)
                        nc.gpsimd.collective_compute(
                            "AllReduce",
                            ALU.add,
                            replica_groups=[[0, 1], [2, 3], [4, 5], [6, 7]],
                            ins=[ar_in[:, :].opt()],
                            outs=[ar_out[:, :].opt()],
                        )
                        r_sb = head.tile([128, MO_CH * BL], f32)
                        nc.sync.dma_start(out=r_sb, in_=ar_out[:, :])
                    else:
                        r_sb = p1_sb
                    h1 = head.tile([128, MO_CH * BL], f16)
                    for mo in range(MO_CH):
                        nc.scalar.activation(
                            out=h1[:, mo * BL:(mo + 1) * BL],
                            in_=r_sb[:, mo * BL:(mo + 1) * BL],
                            func=AF.Relu,
                            bias=b1T_sb[:, mo:mo + 1],
                        )
                    ps2 = head_ps.tile([128, MT_CH * BL], f32)
                    for mt in range(MT_CH):
                        for ko in range(MO_CH):
                            nc.tensor.matmul(
                                ps2[:, mt * BL:(mt + 1) * BL],
                                lhsT=w2t_sb[:, ko, mt * 128:(mt + 1) * 128],
                                rhs=h1[:, ko * BL:(ko + 1) * BL],
                                start=(ko == 0), stop=(ko == MO_CH - 1),
                            )
                    lg_sb = head.tile([128, MT_CH * BL], f32)
                    for mt in range(MT_CH):
                        nc.vector.tensor_scalar(
                            out=lg_sb[:, mt * BL:(mt + 1) * BL],
                            in0=ps2[:, mt * BL:(mt + 1) * BL],
                            scalar1=b2T_sb[:, mt:mt + 1], scalar2=None,
                            op0=ALU.add,
                        )
                    nc.sync.dma_start(out=out_logits[:, :], in_=lg_sb)

    nc.compile()
    return nc


def _tile_kxg(w, n_k):
    """[G, K] weight (already permuted rows) -> [n_k, 128, G] fp16 with
    out[k, kk, g] = w[g, k*128+kk]."""
    wt = w.T.astype(np.float32)  # [K, G]
    return np.ascontiguousarray(
        wt.reshape(n_k, 128, -1)).astype(np.float16)


def prep_core_inputs(x_dir, wih_p, whh_p, bias_p, sc_tb, fc1_w, fc1_b,
                     fc2_w, fc2_b, direction):
    """Build the per-core input map. x_dir [BL, T, E] f32 (already reversed for
    bwd), weights already gate-permuted."""
    import ml_dtypes

    fp8 = os.environ.get("LSTM_FP8", "0") == "1"
    ins = {}
    # xt [E_CH, 128, T*BL]: xt[k][kk][t*BL+b] = x_dir[b,t,k*128+kk]
    xtt = x_dir.transpose(2, 1, 0).reshape(E_CH, 128, T * BL)
    ins["xt"] = np.ascontiguousarray(xtt).astype(np.float16)
    ins["wih"] = _tile_kxg(wih_p, E_CH)
    if fp8:
        wt = whh_p.T.astype(np.float32)  # [K, G]
        w8 = wt.reshape(K_CH // 2, 2, 128, G).transpose(0, 2, 1, 3)
        ins["whh8"] = np.ascontiguousarray(w8).astype(ml_dtypes.float8_e4m3)
    else:
        ins["whh"] = _tile_kxg(whh_p, K_CH)
    br = np.zeros((128, G), np.float16)
    br[0, :] = bias_p.astype(np.float16)
    ins["bias_row"] = br
    on = np.zeros((128, BL), np.float16)
    on[0, :] = 1.0
    ins["ones16"] = on
    # sc [128, T, BL] replicated over partitions
    ins["sc"] = np.broadcast_to(
        sc_tb.astype(np.float16)[None, :, :], (128, T, BL)).copy()
    w1d = fc1_w[:, direction * H:(direction + 1) * H]  # [OH, H]
    ins["w1t"] = _tile_kxg(w1d, K_CH)
    ins["b1T"] = np.ascontiguousarray(
        fc1_b.reshape(MO_CH, 128).T).astype(np.float32)
    ins["w2t"] = _tile_kxg(fc2_w, MO_CH)
    ins["b2T"] = np.ascontiguousarray(
        fc2_b.reshape(MT_CH, 128).T).astype(np.float32)
    return ins


_NC_CACHE = {}
LAST_RESULT = None


def make_in_maps(x, lengths, attn_w, Wih_f, Whh_f, bih_f, bhh_f,
                 Wih_b, Whh_b, bih_b, bhh_b, fc1_w, fc1_b, fc2_w, fc2_b):
    # softmax over attn_w (host glue, exact fp32 as in reference)
    aw = attn_w - attn_w.max()
    e = np.exp(aw)
    scores = (e / e.sum()).astype(np.float32)  # [T]

    tr = np.arange(T)
    # forward sc: sc_f[t, b] = scores[t] * (t < len_b)
    # backward sc: sc_b[tau, b] = scores[len_b-1-tau] * (tau < len_b)
    in_maps = []
    for g in range(4):
        bsl = slice(g * BL, (g + 1) * BL)
        xg = x[bsl]                      # [BL, T, E]
        lg = lengths[bsl]                # [BL]
        mask = tr[:, None] < lg[None, :]  # [T, BL]
        sc_f = scores[:, None] * mask
        idx = np.clip(lg[None, :] - 1 - tr[:, None], 0, T - 1)  # [T, BL]
        sc_b = scores[idx] * mask
        # x reversed per sequence (zeros past length)
        idxc = np.clip(lg[:, None] - 1 - tr[None, :], 0, T - 1)  # [BL, T]
        xrev = np.take_along_axis(xg, idxc[:, :, None], axis=1)
        xrev = xrev * mask.T[:, :, None]

        bias_f = (bih_f + bhh_f)[_GPERM].astype(np.float32)
        bias_b = (bih_b + bhh_b)[_GPERM].astype(np.float32)
        in_maps.append(prep_core_inputs(
            xg, Wih_f[_GPERM], Whh_f[_GPERM], bias_f, sc_f,
            fc1_w, fc1_b, fc2_w, fc2_b, 0))
        in_maps.append(prep_core_inputs(
            xrev, Wih_b[_GPERM], Whh_b[_GPERM], bias_b, sc_b,
            fc1_w, fc1_b, fc2_w, fc2_b, 1))
    return in_maps


def kernel(x, lengths, attn_w, Wih_f, Whh_f, bih_f, bhh_f,
           Wih_b, Whh_b, bih_b, bhh_b, fc1_w, fc1_b, fc2_w, fc2_b):
    x = np.asarray(x, np.float32)
    lengths = np.asarray(lengths, np.int32)
    attn_w = np.asarray(attn_w, np.float32)
    use_collective = os.environ.get("LSTM_NO_COLLECTIVE", "0") != "1"
    fp8 = os.environ.get("LSTM_FP8", "0") == "1"

    key = (T, use_collective, fp8)
    if key not in _NC_CACHE:
        _NC_CACHE[key] = build_nc(T, use_collective, fp8=fp8)
    nc = _NC_CACHE[key]

    in_maps = make_in_maps(x, lengths, attn_w, Wih_f, Whh_f, bih_f, bhh_f,
                           Wih_b, Whh_b, bih_b, bhh_b, fc1_w, fc1_b,
                           fc2_w, fc2_b)
    tr = np.arange(T)
    trace = os.environ.get("LSTM_TRACE", "0") == "1"
    res = run_bass_kernel_spmd(nc, in_maps, list(range(8)), trace=trace)
    results = res.results
    global LAST_RESULT
    LAST_RESULT = res

    out = np.empty((B, T), np.float32)
    for g in range(4):
        if use_collective:
            lt = results[2 * g]["out_logits"]  # [128, MT_CH*BL]
            lg_out = lt.reshape(128, MT_CH, BL).transpose(2, 1, 0).reshape(BL, T)
        else:
            # host head from pooled partials
            pf = results[2 * g]["out_pooled"]
            pb = results[2 * g + 1]["out_pooled"]
            pooled = np.concatenate(
                [pf.reshape(128, K_CH, BL).transpose(2, 1, 0).reshape(BL, H),
                 pb.reshape(128, K_CH, BL).transpose(2, 1, 0).reshape(BL, H)],
                axis=1)
            h1 = np.maximum(pooled @ fc1_w.T + fc1_b, 0.0)
            lg_out = h1 @ fc2_w.T + fc2_b
        out[g * BL:(g + 1) * BL] = lg_out
    tmask = tr[None, :] < lengths[:, None]
    return np.where(tmask, out, np.float32(-1e30)).astype(np.float32)


# revision 26
# speedup vs baseline: 1.5710x; 1.0349x over previous
"""Trainium2 Bass kernel for bidirectional masked-LSTM + attention pooling + FC head.

Problem (hardcoded shapes): B=64, T=512, E=256, H=512, OH=1024.
  - x [B,T,E] f32, lengths [B] i32, attn_w [T] f32
  - per-direction LSTM weights Wih [4H,E], Whh [4H,H], biases [4H]
  - fc1 [OH,2H]+[OH], fc2 [T,OH]+[T]
  - out: logits [B,T] f32, padded positions = -1e30

Sharding: 8 cores = 4 batch groups (16 seqs) x 2 directions. Each core runs one
direction's full 512-step recurrence for its 16 sequences.

Key structure (v2): no input-projection phase and no DRAM xp spill. x stays in
SBUF; per step the gate pre-activations accumulate in PSUM from three matmul
series that share an accumulation group per 16-col slice:
  bias (rank-1 matmul vs a ones row) -> Wih.x_t (2 k-chunks) -> Whh.h (4 k-chunks)
The bias+Wih matmuls do not depend on h, so they execute during the previous
step's elementwise tail; only the 64 Whh matmuls sit on the serial h->h chain.
Gates use three separate PSUM tiles (g / i,f / o in permuted order [i,f,o,g])
so tanh(g) can start while the i,f,o matmuls still run, and the activations
read PSUM directly. Attention pooling is a masked weighted accumulate on the
Pool engine (per-(t,b) scale table precomputed on host, which also implements
sequence reversal for the backward direction). The FC head runs on every core;
forward/backward pooled partials are combined with a pairwise AllReduce.

Layouts (per core):
  h "hidden-tiled" [128, K_CH*16]: h[b, hid] at partition hid%128, col (hid//128)*16+b.
  PSUM gate tiles: col j*16+b holds gate chunk m (m=j+offset), partition = gate%128.
"""

import os

import numpy as np

import concourse.bass as bass
import concourse.tile as tile
from concourse import bacc, mybir
from concourse.bass_utils import run_bass_kernel_spmd

B, T, E, H, OH = 64, 512, 256, 512, 1024
G = 4 * H          # 2048 gates
BL = 16            # batch per core
M_CH = G // 128    # 16 gate chunks
K_CH = H // 128    # 4 hidden chunks
E_CH = E // 128    # 2 input chunks
MO_CH = OH // 128  # 8
MT_CH = T // 128   # 4

f32 = mybir.dt.float32
f16 = mybir.dt.float16
f8 = mybir.dt.float8e4
AF = mybir.ActivationFunctionType
ALU = mybir.AluOpType
DR = mybir.MatmulPerfMode.DoubleRow

# gate permutation: torch order [i,f,g,o] -> kernel order [i,f,o,g]
# perm[new_pos] = old_index  (applied to rows of Wih/Whh and bias)
_GPERM = np.concatenate([
    np.arange(0, H),          # i
    np.arange(H, 2 * H),      # f
    np.arange(3 * H, 4 * H),  # o
    np.arange(2 * H, 3 * H),  # g
])

# (psum tag, m-chunk list) per gate group; emission order of the Whh series is
# g first so tanh(g) overlaps the remaining matmuls.
_GROUPS = [
    ("Pg", list(range(12, 16))),   # g
    ("Pif", list(range(0, 8))),    # i, f
    ("Po", list(range(8, 12))),    # o
]


def _bc_free(ap, reps, width):
    """AP that broadcasts a [P, width] slice to [P, reps, width] via stride-0."""
    return bass.AP(
        tensor=ap.tensor,
        offset=ap.offset,
        ap=[ap.ap[0], [0, reps]] + list(ap.ap[1:]),
    )


def build_nc(t_steps=T, use_collective=True, fp8=False):
    nc = bacc.Bacc("TRN2", target_bir_lowering=False, num_devices=8)

    # ---- DRAM parameters (per-core payloads prepared on host) ----
    xt = nc.declare_dram_parameter("xt", [E_CH, 128, T * BL], f16, isOutput=False)
    wih = nc.declare_dram_parameter("wih", [E_CH, 128, G], f16, isOutput=False)
    if fp8:
        # DoubleRow layout: whh8[cc][p][i][g] = Whh[g, cc*256 + i*128 + p]
        whh8 = nc.declare_dram_parameter("whh8", [K_CH // 2, 128, 2, G], f8,
                                         isOutput=False)
    else:
        whh = nc.declare_dram_parameter("whh", [K_CH, 128, G], f16,
                                        isOutput=False)
    bias_row = nc.declare_dram_parameter("bias_row", [128, G], f16, isOutput=False)
    ones16 = nc.declare_dram_parameter("ones16", [128, BL], f16, isOutput=False)
    sc = nc.declare_dram_parameter("sc", [128, T, BL], f16, isOutput=False)
    w1t = nc.declare_dram_parameter("w1t", [K_CH, 128, OH], f16, isOutput=False)
    b1T = nc.declare_dram_parameter("b1T", [128, MO_CH], f32, isOutput=False)
    w2t = nc.declare_dram_parameter("w2t", [MO_CH, 128, T], f16, isOutput=False)
    b2T = nc.declare_dram_parameter("b2T", [128, MT_CH], f32, isOutput=False)

    out_logits = nc.declare_dram_parameter("out_logits", [128, MT_CH * BL], f32,
                                           isOutput=True)
    out_pooled = nc.declare_dram_parameter("out_pooled", [128, K_CH * BL], f32,
                                           isOutput=True)
    out_h = nc.declare_dram_parameter("out_h", [128, K_CH * BL], f32,
                                      isOutput=True)
    out_c = nc.declare_dram_parameter("out_c", [128, K_CH * BL], f32,
                                      isOutput=True)

    ar_in = nc.dram_tensor("ar_in", [128, MO_CH * BL], f32)
    ar_out = nc.dram_tensor("ar_out", [128, MO_CH * BL], f32)

    with tile.TileContext(nc) as tc:
        with tc.tile_pool(name="const", bufs=1) as const_pool:
            if fp8:
                whh8_sb = const_pool.tile([128, K_CH // 2, 2, G], f8)
                for cc in range(K_CH // 2):
                    nc.sync.dma_start(out=whh8_sb[:, cc, :, :], in_=whh8[cc])
            else:
                whh_sb = const_pool.tile([128, K_CH, G], f16)
                for k in range(K_CH):
                    nc.sync.dma_start(out=whh_sb[:, k, :], in_=whh[k])
            wih_sb = const_pool.tile([128, E_CH, G], f16)
            for k in range(E_CH):
                nc.sync.dma_start(out=wih_sb[:, k, :], in_=wih[k])
            xt_sb = const_pool.tile([128, E_CH, T * BL], f16)
            for k in range(E_CH):
                nc.sync.dma_start(out=xt_sb[:, k, :], in_=xt[k])
            biask_sb = const_pool.tile([128, M_CH, 128], f16)
            nc.sync.dma_start(out=biask_sb[:, :, :], in_=bias_row[:, :])
            ones_sb = const_pool.tile([128, BL], f16)
            nc.sync.dma_start(out=ones_sb, in_=ones16[:, :])
            sc_sb = const_pool.tile([128, T, BL], f16)
            nc.sync.dma_start(out=sc_sb, in_=sc[:, :, :])
            w1t_sb = const_pool.tile([128, K_CH, OH], f16)
            for k in range(K_CH):
                nc.sync.dma_start(out=w1t_sb[:, k, :], in_=w1t[k])
            b1T_sb = const_pool.tile([128, MO_CH], f32)
            nc.sync.dma_start(out=b1T_sb, in_=b1T[:, :])
            w2t_sb = const_pool.tile([128, MO_CH, T], f16)
            for k in range(MO_CH):
                nc.sync.dma_start(out=w2t_sb[:, k, :], in_=w2t[k])
            b2T_sb = const_pool.tile([128, MT_CH], f32)
            nc.sync.dma_start(out=b2T_sb, in_=b2T[:, :])

            # ---- recurrence ----
            with tc.tile_pool(name="state", bufs=1) as state_pool:
                h_sb = state_pool.tile([128, K_CH * BL], f16)
                c_sb = state_pool.tile([128, K_CH * BL], f32)
                acc = state_pool.tile([128, K_CH * BL], f32)
                nc.vector.memset(h_sb, 0.0)
                nc.vector.memset(c_sb, 0.0)
                nc.gpsimd.memset(acc, 0.0)
                if fp8:
                    # fp8 copy of h for the DoubleRow Whh matmul; col order
                    # (k,b) with k = cc*2+i matches the (cc,i,b) nesting.
                    h8_sb = state_pool.tile([128, K_CH // 2, 2, BL], f8)
                    nc.vector.memset(h8_sb, 0.0)

                with tc.tile_pool(name="rec_ps", bufs=2, space="PSUM") as rec_ps, \
                     tc.tile_pool(name="work", bufs=2) as work:
                    from contextlib import ExitStack
                    for t in range(t_steps):
                        # Gate each step's instructions at virtual time t ms:
                        # the compile-time list scheduler otherwise hoists
                        # future X-phases ahead of the h-critical Whh
                        # matmuls (its cost model over-estimates the step,
                        # which becomes self-fulfilling in the static order).
                        step_gate = ExitStack()
                        step_gate.enter_context(tc.tile_wait_until(ms=t))
                        # X tile: bias + Wih.x_t, complete per-slice groups,
                        # one PSUM bank for all 16 slices (regions g|if|o).
                        # No h dependency -> these execute during the previous
                        # step's tail. PSUM accumulation groups must be
                        # contiguous in the PE stream, hence separate X/H
                        # tiles combined by a vector add in the tail.
                        xfull = rec_ps.tile([128, M_CH * BL], f32, tag="X")
                        xtiles = {}
                        off = 0
                        for tag, ms in _GROUPS:
                            xt_ = xfull[:, off * BL:(off + len(ms)) * BL]
                            off += len(ms)
                            xtiles[tag] = xt_
                            for j, m in enumerate(ms):
                                sl = xt_[:, j * BL:(j + 1) * BL]
                                nc.tensor.matmul(
                                    sl, lhsT=biask_sb[:, m, :], rhs=ones_sb,
                                    start=True, stop=False)
                                for k in range(E_CH):
                                    nc.tensor.matmul(
                                        sl,
                                        lhsT=wih_sb[:, k, m * 128:(m + 1) * 128],
                                        rhs=xt_sb[:, k, t * BL:(t + 1) * BL],
                                        start=False, stop=(k == E_CH - 1))
                        # DVE can read only one PSUM input per op: evacuate X
                        # to SBUF (off the critical path, Act has slack).
                        xe = work.tile([128, M_CH * BL], f16, tag="xe")
                        nc.scalar.activation(out=xe, in_=xfull, func=AF.Identity)
                        xtiles = {"Pg": xe[:, 0:4 * BL],
                                  "Pif": xe[:, 4 * BL:12 * BL],
                                  "Po": xe[:, 12 * BL:16 * BL]}
                        # H tiles: Whh.h, per-slice groups; i,f chunks first —
                        # sigma(i,f) is the longest pole of the tail chain.
                        htiles = {}
                        for tag, ms in [_GROUPS[1], _GROUPS[0], _GROUPS[2]]:
                            ht_ = rec_ps.tile([128, len(ms) * BL], f32,
                                              tag="H" + tag)
                            htiles[tag] = ht_
                            for j, m in enumerate(ms):
                                sl = ht_[:, j * BL:(j + 1) * BL]
                                if fp8:
                                    for cc in range(K_CH // 2):
                                        nc.tensor.matmul(
                                            sl,
                                            lhsT=whh8_sb[:, cc, :,
                                                         m * 128:(m + 1) * 128],
                                            rhs=h8_sb[:, cc, :, :],
                                            start=(cc == 0),
                                            stop=(cc == K_CH // 2 - 1),
                                            perf_mode=DR,
                                        )
                                else:
                                    for k in range(K_CH):
                                        nc.tensor.matmul(
                                            sl,
                                            lhsT=whh_sb[:, k,
                                                        m * 128:(m + 1) * 128],
                                            rhs=h_sb[:, k * BL:(k + 1) * BL],
                                            start=(k == 0),
                                            stop=(k == K_CH - 1),
                                        )
                        gA = work.tile([128, 4 * BL], f16, tag="gA")
                        nc.vector.tensor_add(out=gA, in0=xtiles["Pg"],
                                             in1=htiles["Pg"])
                        tg = work.tile([128, 4 * BL], f16, tag="tg")
                        nc.scalar.activation(out=tg, in_=gA, func=AF.Tanh)
                        ifA = work.tile([128, 8 * BL], f16, tag="ifA")
                        nc.vector.tensor_add(out=ifA, in0=xtiles["Pif"],
                                             in1=htiles["Pif"])
                        sif = work.tile([128, 8 * BL], f16, tag="sif")
                        nc.scalar.activation(out=sif, in_=ifA, func=AF.Sigmoid)
                        oA = work.tile([128, 4 * BL], f16, tag="oA")
                        nc.vector.tensor_add(out=oA, in0=xtiles["Po"],
                                             in1=htiles["Po"])
                        so = work.tile([128, 4 * BL], f16, tag="so")
                        nc.scalar.activation(out=so, in_=oA, func=AF.Sigmoid)
                        t1 = work.tile([128, 4 * BL], f16, tag="t1")
                        nc.vector.tensor_mul(out=t1, in0=sif[:, 0:4 * BL], in1=tg)
                        t2 = work.tile([128, 4 * BL], f32, tag="t2")
                        nc.vector.tensor_mul(out=t2, in0=sif[:, 4 * BL:8 * BL],
                                             in1=c_sb)
                        nc.vector.tensor_add(out=c_sb, in0=t1, in1=t2)
                        tch = work.tile([128, 4 * BL], f16, tag="tch")
                        nc.scalar.activation(out=tch, in_=c_sb, func=AF.Tanh)
                        if fp8:
                            nc.vector.tensor_mul(out=h8_sb, in0=so, in1=tch)
                        nc.vector.tensor_mul(out=h_sb, in0=so, in1=tch)

                        pt = work.tile([128, 4 * BL], f32, tag="pt")
                        nc.gpsimd.tensor_mul(
                            out=pt, in0=h_sb,
                            in1=_bc_free(sc_sb[:, t, :], K_CH, BL),
                        )
                        nc.gpsimd.tensor_add(out=acc, in0=acc, in1=pt)
                        step_gate.close()

                # ---- head ----
                with tc.tile_pool(name="head", bufs=1) as head, \
                     tc.tile_pool(name="head_ps", bufs=1, space="PSUM") as head_ps:
                    nc.sync.dma_start(out=out_pooled[:, :], in_=acc)
                    hf = head.tile([128, K_CH * BL], f32)
                    nc.vector.tensor_copy(out=hf, in_=h_sb)
                    nc.sync.dma_start(out=out_h[:, :], in_=hf)
                    nc.sync.dma_start(out=out_c[:, :], in_=c_sb)
                    acch = head.tile([128, K_CH * BL], f16)
                    nc.vector.tensor_copy(out=acch, in_=acc)
                    ps1 = head_ps.tile([128, MO_CH * BL], f32)
                    for mo in range(MO_CH):
                        for k in range(K_CH):
                            nc.tensor.matmul(
                                ps1[:, mo * BL:(mo + 1) * BL],
                                lhsT=w1t_sb[:, k, mo * 128:(mo + 1) * 128],
                                rhs=acch[:, k * BL:(k + 1) * BL],
                                start=(k == 0), stop=(k == K_CH - 1),
                            )
                    p1_sb = head.tile([128, MO_CH * BL], f32)
                    nc.vector.tensor_copy(out=p1_sb, in_=ps1)
                    if use_collective:
                        nc.sync.dma_start(out=ar_in[:, :], in_=p1_sb)
                        nc.gpsimd.collective_compute(
                            "AllReduce",
                            ALU.add,
                            replica_groups=[[0, 1], [2, 3], [4, 5], [6, 7]],
                            ins=[ar_in[:, :].opt()],
                            outs=[ar_out[:, :].opt()],
                        )
                        r_sb = head.tile([128, MO_CH * BL], f32)
                        nc.sync.dma_start(out=r_sb, in_=ar_out[:, :])
                    else:
                        r_sb = p1_sb
                    h1 = head.tile([128, MO_CH * BL], f16)
                    for mo in range(MO_CH):
                        nc.scalar.activation(
                            out=h1[:, mo * BL:(mo + 1) * BL],
                            in_=r_sb[:, mo * BL:(mo + 1) * BL],
                            func=AF.Relu,
                            bias=b1T_sb[:, mo:mo + 1],
                        )
                    ps2 = head_ps.tile([128, MT_CH * BL], f32)
                    for mt in range(MT_CH):
                        for ko in range(MO_CH):
                            nc.tensor.matmul(
                                ps2[:, mt * BL:(mt + 1) * BL],
                                lhsT=w2t_sb[:, ko, mt * 128:(mt + 1) * 128],
                                rhs=h1[:, ko * BL:(ko + 1) * BL],
                                start=(ko == 0), stop=(ko == MO_CH - 1),
                            )
                    lg_sb = head.tile([128, MT_CH * BL], f32)
                    for mt in range(MT_CH):
                        nc.vector.tensor_scalar(
                            out=lg_sb[:, mt * BL:(mt + 1) * BL],
                            in0=ps2[:, mt * BL:(mt + 1) * BL],
                            scalar1=b2T_sb[:, mt:mt + 1], scalar2=None,
                            op0=ALU.add,
                        )
                    nc.sync.dma_start(out=out_logits[:, :], in_=lg_sb)

    nc.compile()
    return nc


def _tile_kxg(w, n_k):
    """[G, K] weight (already permuted rows) -> [n_k, 128, G] fp16 with
    out[k, kk, g] = w[g, k*128+kk]."""
    wt = w.T.astype(np.float32)  # [K, G]
    return np.ascontiguousarray(
        wt.reshape(n_k, 128, -1)).astype(np.float16)


def prep_core_inputs(x_dir, wih_p, whh_p, bias_p, sc_tb, fc1_w, fc1_b,
                     fc2_w, fc2_b, direction):
    """Build the per-core input map. x_dir [BL, T, E] f32 (already reversed for
    bwd), weights already gate-permuted."""
    import ml_dtypes

    fp8 = os.environ.get("LSTM_FP8", "0") == "1"
    ins = {}
    # xt [E_CH, 128, T*BL]: xt[k][kk][t*BL+b] = x_dir[b,t,k*128+kk]
    xtt = x_dir.transpose(2, 1, 0).reshape(E_CH, 128, T * BL)
    ins["xt"] = np.ascontiguousarray(xtt).astype(np.float16)
    ins["wih"] = _tile_kxg(wih_p, E_CH)
    if fp8:
        wt = whh_p.T.astype(np.float32)  # [K, G]
        w8 = wt.reshape(K_CH // 2, 2, 128, G).transpose(0, 2, 1, 3)
        ins["whh8"] = np.ascontiguousarray(w8).astype(ml_dtypes.float8_e4m3)
    else:
        ins["whh"] = _tile_kxg(whh_p, K_CH)
    br = np.zeros((128, G), np.float16)
    br[0, :] = bias_p.astype(np.float16)
    ins["bias_row"] = br
    on = np.zeros((128, BL), np.float16)
    on[0, :] = 1.0
    ins["ones16"] = on
    # sc [128, T, BL] replicated over partitions
    ins["sc"] = np.broadcast_to(
        sc_tb.astype(np.float16)[None, :, :], (128, T, BL)).copy()
    w1d = fc1_w[:, direction * H:(direction + 1) * H]  # [OH, H]
    ins["w1t"] = _tile_kxg(w1d, K_CH)
    ins["b1T"] = np.ascontiguousarray(
        fc1_b.reshape(MO_CH, 128).T).astype(np.float32)
    ins["w2t"] = _tile_kxg(fc2_w, MO_CH)
    ins["b2T"] = np.ascontiguousarray(
        fc2_b.reshape(MT_CH, 128).T).astype(np.float32)
    return ins


_NC_CACHE = {}
LAST_RESULT = None


def make_in_maps(x, lengths, attn_w, Wih_f, Whh_f, bih_f, bhh_f,
                 Wih_b, Whh_b, bih_b, bhh_b, fc1_w, fc1_b, fc2_w, fc2_b):
    # softmax over attn_w (host glue, exact fp32 as in reference)
    aw = attn_w - attn_w.max()
    e = np.exp(aw)
    scores = (e / e.sum()).astype(np.float32)  # [T]

    tr = np.arange(T)
    # forward sc: sc_f[t, b] = scores[t] * (t < len_b)
    # backward sc: sc_b[tau, b] = scores[len_b-1-tau] * (tau < len_b)
    in_maps = []
    for g in range(4):
        bsl = slice(g * BL, (g + 1) * BL)
        xg = x[bsl]                      # [BL, T, E]
        lg = lengths[bsl]                # [BL]
        mask = tr[:, None] < lg[None, :]  # [T, BL]
        sc_f = scores[:, None] * mask
        idx = np.clip(lg[None, :] - 1 - tr[:, None], 0, T - 1)  # [T, BL]
        sc_b = scores[idx] * mask
        # x reversed per sequence (zeros past length)
        idxc = np.clip(lg[:, None] - 1 - tr[None, :], 0, T - 1)  # [BL, T]
        xrev = np.take_along_axis(xg, idxc[:, :, None], axis=1)
        xrev = xrev * mask.T[:, :, None]

        bias_f = (bih_f + bhh_f)[_GPERM].astype(np.float32)
        bias_b = (bih_b + bhh_b)[_GPERM].astype(np.float32)
        in_maps.append(prep_core_inputs(
            xg, Wih_f[_GPERM], Whh_f[_GPERM], bias_f, sc_f,
            fc1_w, fc1_b, fc2_w, fc2_b, 0))
        in_maps.append(prep_core_inputs(
            xrev, Wih_b[_GPERM], Whh_b[_GPERM], bias_b, sc_b,
            fc1_w, fc1_b, fc2_w, fc2_b, 1))
    return in_maps


def kernel(x, lengths, attn_w, Wih_f, Whh_f, bih_f, bhh_f,
           Wih_b, Whh_b, bih_b, bhh_b, fc1_w, fc1_b, fc2_w, fc2_b):
    x = np.asarray(x, np.float32)
    lengths = np.asarray(lengths, np.int32)
    attn_w = np.asarray(attn_w, np.float32)
    use_collective = os.environ.get("LSTM_NO_COLLECTIVE", "0") != "1"
    fp8 = os.environ.get("LSTM_FP8", "0") == "1"

    key = (T, use_collective, fp8)
    if key not in _NC_CACHE:
        _NC_CACHE[key] = build_nc(T, use_collective, fp8=fp8)
    nc = _NC_CACHE[key]

    in_maps = make_in_maps(x, lengths, attn_w, Wih_f, Whh_f, bih_f, bhh_f,
                           Wih_b, Whh_b, bih_b, bhh_b, fc1_w, fc1_b,
                           fc2_w, fc2_b)
    tr = np.arange(T)
    trace = os.environ.get("LSTM_TRACE", "0") == "1"
    res = run_bass_kernel_spmd(nc, in_maps, list(range(8)), trace=trace)
    results = res.results
    global LAST_RESULT
    LAST_RESULT = res

    out = np.empty((B, T), np.float32)
    for g in range(4):
        if use_collective:
            lt = results[2 * g]["out_logits"]  # [128, MT_CH*BL]
            lg_out = lt.reshape(128, MT_CH, BL).transpose(2, 1, 0).reshape(BL, T)
        else:
            # host head from pooled partials
            pf = results[2 * g]["out_pooled"]
            pb = results[2 * g + 1]["out_pooled"]
            pooled = np.concatenate(
                [pf.reshape(128, K_CH, BL).transpose(2, 1, 0).reshape(BL, H),
                 pb.reshape(128, K_CH, BL).transpose(2, 1, 0).reshape(BL, H)],
                axis=1)
            h1 = np.maximum(pooled @ fc1_w.T + fc1_b, 0.0)
            lg_out = h1 @ fc2_w.T + fc2_b
        out[g * BL:(g + 1) * BL] = lg_out
    tmask = tr[None, :] < lengths[:, None]
    return np.where(tmask, out, np.float32(-1e30)).astype(np.float32)
